# revision 36
# baseline (speedup 1.0000x reference)
"""Trainium2 Bass kernel v6 for nn_Attention (B=1, N=4096, DIM=768, HEADS=12).

Sharding: queries 8-way (512/core, all 12 heads); K/V for pairs 1-5 (K)
and 2-5 (V) exchanged via per-pair AllGathers (CC engine does a ~21+30-70us
init, then ~15us per 1MB gather); pair-0 K and pair-0/1 V are computed
locally from a full x^T copy while the CC engine warms up.

Main structure (vs the 436us v4 baseline):
- Single shared PSUM plan for the entire kernel: ring pool of
  2x[128,3,512] f32 slots (6 banks) + pO1/pO2 AV accumulators (2 banks)
  = 16KB exactly. Every matmul output (scores, projections, output
  proj) allocates from the ring -> no pool-transition drain barriers
  (v4 serialized phases through PSUM reuse; first exp was at 137us,
  now ~45us).
- Row-packed score matmuls: kt/qT pack a head pair on 128 partitions
  (h1 dims 0-63, h2 64-127); two K=64 matmuls via tile_position
  (auto-derived) issue ~30ns apart and share the array row groups.
- exp batched to [128,3,512] PSUM slots (F=1536): 132 ACTIVATEs of
  ~1.54us ~= 203us ACT busy; ACT is the steady-state bottleneck, with
  the (power-throttled ~1.95GHz) PE at ~90%+ occupancy underneath.
- Softmax denominators ride the AV matmuls as a ones-column of V
  (M=65); normalization = DVE copy of the denominator row (64->0),
  reciprocal_approx_fast, gpsimd partition_broadcast, DVE multiply
  straight out of PSUM.
- All projection work (kfull0 chunks, V01 N=256 chunks, sharded
  K/V + AG staging, qproj 1-5) is emitted as deferred closures drained
  by per-slot hooks inside the pair-0/1 attention windows, keeping
  producers ahead of consumers (the dep tracker only links reads to
  already-emitted writers). Gather loads for pair j+1 are emitted
  mid-pair j after their AG producers.
- Output projection split: pairs 0-2 (+bias) pre-accumulated into SBUF
  during pair 3, pairs 3-4 during pair 5, so the tail only computes
  pair 5's contribution.
- DMAs: x16 quarters on the sync queue; xc16/wq/wv on scalar; wk/wo on
  gpsimd; the pair-0-critical column slices (wk/wq cols 0:128, wv
  0:256) load first. PE warmer matmuls open the HAM clock gate before
  the first projections. Pair-0's first eight score slots are emitted
  under tc.high_priority() so the static scheduler places them at PE
  queue position ~53 instead of ~145 (behind DMA-gated ksh/vsh work,
  which head-of-line blocked the first exp). Output stores are split
  into row-halves on two queues.

Measured: ~330-340us (from 436us), rel err 3.2e-3.
"""

import os
import sys
from contextlib import ExitStack

import numpy as np

sys.path.insert(0, "/opt/trn_rl_repo")

import concourse.bass as bass  # noqa: E402
import concourse.tile as tile  # noqa: E402
from concourse import bacc, mybir  # noqa: E402
from concourse.bass_utils import run_bass_kernel_spmd  # noqa: E402

N_CORES = 8
DIM = 768
HEADS = 12
SEQ = 4096
DHEAD = 64
NQ = SEQ // N_CORES  # 512 queries per core
NPAIRS = HEADS // 2  # 6 head pairs
KT = DIM // 128  # 6 contraction tiles
NKB = SEQ // 128  # 32 key blocks
F32 = mybir.dt.float32
F16 = mybir.dt.float16
BF16 = mybir.dt.bfloat16
Exp = mybir.ActivationFunctionType.Exp

_CACHE = {}


def _build():
    nc = bacc.Bacc("TRN2", target_bir_lowering=False, debug=False, num_devices=N_CORES)

    xc16 = nc.dram_tensor("xc16", [KT, 128, NQ], F16, kind="ExternalInput").ap()
    xT16 = nc.dram_tensor("xT16", [KT, 128, SEQ], F16, kind="ExternalInput").ap()
    wq = nc.dram_tensor("wq", [KT, 128, DIM], F16, kind="ExternalInput").ap()
    wk = nc.dram_tensor("wk", [KT, 128, DIM], F16, kind="ExternalInput").ap()
    wv = nc.dram_tensor("wv", [KT, 128, DIM], F16, kind="ExternalInput").ap()
    wo = nc.dram_tensor("wo", [NPAIRS, 128, DIM], BF16, kind="ExternalInput").ap()
    bo = nc.dram_tensor("bo", [DIM], F32, kind="ExternalInput").ap()
    out = nc.dram_tensor("out", [NQ, DIM], F32, kind="ExternalOutput").ap()

    # collective bounce buffers: K for pairs 1-5, V for pairs 2-5
    agk_in = {p: nc.dram_tensor(f"agk_in{p}", [128, NQ], F16).ap() for p in range(1, 6)}
    agk_out = {
        p: nc.dram_tensor(f"agk_out{p}", [N_CORES, 128, NQ], F16, addr_space="Shared").ap()
        for p in range(1, 6)
    }
    agv_in = {
        p: nc.dram_tensor(f"agv_in{p}", [128, 4, 2, DHEAD + 1], BF16).ap()
        for p in range(2, 6)
    }
    agv_out = {
        p: nc.dram_tensor(
            f"agv_out{p}", [N_CORES, 128, 4, 2, DHEAD + 1], BF16, addr_space="Shared"
        ).ap()
        for p in range(2, 6)
    }
    groups = [list(range(N_CORES))]

    with ExitStack() as ctx:
        tc = ctx.enter_context(tile.TileContext(nc))

        persist = ctx.enter_context(tc.tile_pool(name="persist", bufs=1))
        ring = ctx.enter_context(tc.tile_pool(name="ring", bufs=2, space="PSUM"))
        psO = ctx.enter_context(tc.tile_pool(name="psO", bufs=1, space="PSUM"))
        evac = ctx.enter_context(tc.tile_pool(name="evac", bufs=3))
        epool = ctx.enter_context(tc.tile_pool(name="epool", bufs=5))
        npool = ctx.enter_context(tc.tile_pool(name="npool", bufs=2))

        # ---- persistent SBUF ----
        qT_sb = [persist.tile([128, NQ], F16, tag=f"qt{p}", name=f"qt{p}") for p in range(NPAIRS)]
        proj_sb = [persist.tile([128, NQ], BF16, tag=f"proj{p}", name=f"proj{p}") for p in range(NPAIRS)]
        ktiles = [persist.tile([128, SEQ], F16, tag=f"ktile{i}", name=f"ktile{i}") for i in range(2)]
        vtiles = [
            persist.tile([128, NKB, 2, DHEAD + 1], BF16, tag=f"vtile{i}", name=f"vtile{i}")
            for i in range(3)
        ]
        xc_sb = [persist.tile([128, NQ], F16, tag=f"xc{k}", name=f"xc{k}") for k in range(KT)]
        x16_sb = [persist.tile([128, SEQ], F16, tag=f"x16_{k}", name=f"x16_{k}") for k in range(KT)]
        wq_sb = [persist.tile([128, DIM], F16, tag=f"wq{k}", name=f"wq{k}") for k in range(KT)]
        wk_sb = [persist.tile([128, DIM], F16, tag=f"wk{k}", name=f"wk{k}") for k in range(KT)]
        wv_sb = [persist.tile([128, DIM], F16, tag=f"wv{k}", name=f"wv{k}") for k in range(KT)]
        wo_sb = [persist.tile([128, DIM], BF16, tag=f"wo{p}", name=f"wo{p}") for p in range(NPAIRS)]
        bias_sb = persist.tile([128, DIM], F32, tag="bias", name="bias")
        accC = [persist.tile([128, DIM], F32, tag=f"accC{qt}", name=f"accC{qt}")
                for qt in range(NQ // 128)]

        # ---- DMAs. The first-exp chain only needs narrow column slices
        # (wq/wk cols 0:128 for pair 0, wv cols 0:256 for V01), so those
        # tiny loads go first; the wide remainders follow. x16 quarters own
        # the sync queue. ----
        for quarter in range(4):
            for k in range(KT):
                nc.sync.dma_start(
                    out=x16_sb[k][:, quarter * 1024:(quarter + 1) * 1024],
                    in_=xT16[k][:, quarter * 1024:(quarter + 1) * 1024],
                )
        for k in range(KT):
            nc.scalar.dma_start(out=xc_sb[k][:], in_=xc16[k])
        for k in range(KT):
            nc.gpsimd.dma_start(out=wk_sb[k][:, 0:128], in_=wk[k][:, 0:128])
        for k in range(KT):
            nc.scalar.dma_start(out=wq_sb[k][:, 0:128], in_=wq[k][:, 0:128])
        for k in range(KT):
            nc.scalar.dma_start(out=wv_sb[k][:, 0:256], in_=wv[k][:, 0:256])
        for k in range(KT):
            nc.gpsimd.dma_start(out=wk_sb[k][:, 128:DIM], in_=wk[k][:, 128:DIM])
        for k in range(KT):
            nc.scalar.dma_start(out=wq_sb[k][:, 128:DIM], in_=wq[k][:, 128:DIM])
        for k in range(KT):
            nc.scalar.dma_start(out=wv_sb[k][:, 256:DIM], in_=wv[k][:, 256:DIM])
        for p in range(NPAIRS):
            nc.gpsimd.dma_start(out=wo_sb[p][:], in_=wo[p])
        bo_b = bass.AP(tensor=bo.tensor, offset=bo.offset, ap=[[0, 128]] + bo.ap)
        nc.gpsimd.dma_start(out=bias_sb[:], in_=bo_b)

        # ones columns for locally-computed V (pairs 0-1 in vtiles 0/1);
        # gathered pairs bring ones through the collective.
        for i in range(2):
            nc.vector.memset(vtiles[i][:, :, :, DHEAD:DHEAD + 1], 1.0)

        # PE warmer: back-to-back matmuls reading the last xc tile (so they
        # run right after its DMA lands, just before qproj0) to open the HAM
        # clock gate (1.2 -> 2.4 GHz) going into the critical projections.
        for w in range(8):
            ps = ring.tile([128, 3, NQ], F32, tag="ps", name=f"warm{w}")
            nc.tensor.matmul(ps[:, 0, :], xc_sb[0][:, 0:128], xc_sb[0][:],
                             start=True, stop=True)

        # ---- sharded K (pairs 1-5) / V (pairs 2-5) + AllGathers ----
        def _kproj(p):
            ps = ring.tile([128, 3, NQ], F32, tag="ps", name=f"psk{p}")
            for k in range(KT):
                nc.tensor.matmul(
                    ps[:, 0, :], wk_sb[k][:, p * 128:(p + 1) * 128], xc_sb[k][:],
                    start=(k == 0), stop=(k == KT - 1),
                )
            kev = evac.tile([128, NQ], F16, tag="kev", name="kev")
            nc.vector.tensor_copy(kev[:], ps[:, 0, :])
            nc.gpsimd.dma_start(out=agk_in[p], in_=kev[:])
            nc.gpsimd.collective_compute(
                "AllGather", mybir.AluOpType.bypass, replica_groups=groups,
                ins=[agk_in[p].opt()], outs=[agk_out[p].opt()],
            )

        def _agv(p):
            nc.gpsimd.collective_compute(
                "AllGather", mybir.AluOpType.bypass, replica_groups=groups,
                ins=[agv_in[p].opt()], outs=[agv_out[p].opt()],
            )

        def _vsh(st):
            ps = ring.tile([128, 3, NQ], F32, tag="ps", name=f"psv{st}")
            for k in range(KT):
                nc.tensor.matmul(
                    ps[:, 0, :], xc_sb[k][:, st * 128:(st + 1) * 128],
                    wv_sb[k][:, 256:DIM], start=(k == 0), stop=(k == KT - 1),
                )
            vev = evac.tile([128, 8, DHEAD + 1], BF16, tag="vev", name="vev")
            nc.vector.tensor_copy(
                vev[:, :, 0:DHEAD],
                ps[:, 0, :].rearrange("p (h d) -> p h d", h=8),
            )
            nc.vector.memset(vev[:, :, DHEAD:DHEAD + 1], 1.0)
            for p in range(2, 6):
                nc.gpsimd.dma_start(
                    out=agv_in[p][:, st, :, :],
                    in_=vev[:, 2 * (p - 2):2 * (p - 2) + 2, :],
                )

        # sharded projections + AG triggers drained inside pair-0's window
        # (the CC engine is busy with its ~45-70us init until then anyway)
        ag_work = [lambda: _kproj(1), lambda: _kproj(2),
                   lambda: _vsh(0), lambda: _vsh(1), lambda: _vsh(2),
                   lambda: _vsh(3), lambda: _agv(2),
                   lambda: _kproj(3), lambda: _agv(3),
                   lambda: _kproj(4), lambda: _agv(4),
                   lambda: _kproj(5), lambda: _agv(5)]

        # ---- Q projections (packed per pair); only pair 0 upfront ----
        def _qproj(p):
            ps = ring.tile([128, 3, NQ], F32, tag="ps", name=f"psq{p}")
            for k in range(KT):
                nc.tensor.matmul(
                    ps[:, 0, :], wq_sb[k][:, p * 128:(p + 1) * 128], xc_sb[k][:],
                    start=(k == 0), stop=(k == KT - 1),
                )
            nc.vector.tensor_copy(qT_sb[p][:], ps[:, 0, :])

        _qproj(0)
        q_work = [(lambda p=p: _qproj(p)) for p in range(1, NPAIRS)]

        # ---- full-seq K pair 0, chunk-deferred into pair-0's window ----
        def _kfull0(ch):
            ps = ring.tile([128, 3, NQ], F32, tag="ps", name=f"pskf0_{ch}")
            for k in range(KT):
                nc.tensor.matmul(
                    ps[:, 0, :], wk_sb[k][:, 0:128],
                    x16_sb[k][:, ch * 512:(ch + 1) * 512],
                    start=(k == 0), stop=(k == KT - 1),
                )
            nc.vector.tensor_copy(ktiles[0][:, ch * 512:(ch + 1) * 512], ps[:, 0, :])

        k0_work = [(lambda ch=ch: _kfull0(ch)) for ch in range(8)]
        k0_work.pop(0)()

        # ---- full-seq V pairs 0-1 (cols 0:256), 2 seq-tiles per chunk ----
        def _v01_chunks():
            chunks = []
            for st0 in range(0, NKB, 2):
                def _do(st0=st0):
                    ps = ring.tile([128, 3, NQ], F32, tag="ps", name=f"psv01_{st0}")
                    for s2 in range(2):
                        for k in range(KT):
                            nc.tensor.matmul(
                                ps[:, s2, 0:256],
                                x16_sb[k][:, (st0 + s2) * 128:(st0 + s2 + 1) * 128],
                                wv_sb[k][:, 0:256],
                                start=(k == 0), stop=(k == KT - 1),
                            )
                    for p in range(2):
                        nc.vector.tensor_copy(
                            vtiles[p][:, st0:st0 + 2, :, 0:DHEAD],
                            ps[:, 0:2, p * 128:(p + 1) * 128].rearrange(
                                "p s (h d) -> p s h d", h=2),
                        )
                chunks.append(_do)
            return chunks

        v01_work = _v01_chunks()  # 16 chunks
        v01_work.pop(0)()

        def _drain(lst, n_total, target):
            while n_total - len(lst) < min(n_total, target):
                lst.pop(0)()

        # pair-0 slot s consumes V01 chunk (3s+2)//4 and kfull0 chunk
        # (3s+2)//8 at the most; stay one ahead. ksh/vsh/AG staging and
        # qproj(1) fill the remaining pair-0 slots.
        def _pair0_pre_slot(s):
            _drain(v01_work, 15, 1 + (3 * (s + 1)) // 4)
            _drain(k0_work, 7, 1 + (3 * (s + 1)) // 8)
            _drain(ag_work, 13, max(min(2, s), s - 4))
            if s == 12:
                _drain(q_work, 5, 1)  # qproj(1)

        def _pair1_pre_slot(s):
            if s == 0:
                _drain(v01_work, 15, 15)
                _drain(k0_work, 7, 7)
                _drain(ag_work, 13, 13)
            _drain(q_work, 5, 1 + max(0, (s - 1) // 4))  # qproj(2-5)

        def _pair2_pre_slot(s):
            if s == 0:
                _drain(q_work, 5, 5)

        # partial output projection (pairs 0-2 + bias) during pair 3
        def _partialC(qt):
            ps = ring.tile([128, 3, NQ], F32, tag="ps", name=f"psc{qt}")
            for p in range(3):
                lhs = proj_sb[p][:, qt * 128:(qt + 1) * 128]
                nc.tensor.matmul(ps[:, 0, :], lhs, wo_sb[p][:, 0:512],
                                 start=(p == 0), stop=(p == 2))
                nc.tensor.matmul(ps[:, 1, 0:256], lhs, wo_sb[p][:, 512:DIM],
                                 start=(p == 0), stop=(p == 2))
            nc.vector.tensor_add(accC[qt][:, 0:512], ps[:, 0, :], bias_sb[:, 0:512])
            nc.vector.tensor_add(accC[qt][:, 512:DIM], ps[:, 1, 0:256],
                                 bias_sb[:, 512:DIM])

        def _pair3_pre_slot(s):
            if s >= 4 and s % 4 == 0 and (s - 4) // 4 < 4:
                _partialC((s - 4) // 4)

        def _partialC2(qt):
            ps = ring.tile([128, 3, NQ], F32, tag="ps", name=f"psc2_{qt}")
            for p in range(3, 5):
                lhs = proj_sb[p][:, qt * 128:(qt + 1) * 128]
                nc.tensor.matmul(ps[:, 0, :], lhs, wo_sb[p][:, 0:512],
                                 start=(p == 3), stop=(p == 4))
                nc.tensor.matmul(ps[:, 1, 0:256], lhs, wo_sb[p][:, 512:DIM],
                                 start=(p == 3), stop=(p == 4))
            nc.vector.tensor_add(accC[qt][:, 0:512], ps[:, 0, :], accC[qt][:, 0:512])
            nc.vector.tensor_add(accC[qt][:, 512:DIM], ps[:, 1, 0:256],
                                 accC[qt][:, 512:DIM])

        def _pair5_pre_slot(s):
            if s >= 4 and s % 4 == 0 and (s - 4) // 4 < 4:
                _partialC2((s - 4) // 4)

        pre_slot_hooks = {0: _pair0_pre_slot, 1: _pair1_pre_slot,
                          2: _pair2_pre_slot, 3: _pair3_pre_slot,
                          5: _pair5_pre_slot}

        # ---- gathered pair loads: kt for pairs 1-5, v for pairs 2-5 ----
        def _load_pair(j):
            kt_dst = ktiles[j % 2]
            nc.sync.dma_start(
                out=kt_dst[:].rearrange("p (c s) -> p c s", c=N_CORES),
                in_=agk_out[j].rearrange("c p s -> p c s"),
            )
            if j >= 2:
                v_dst = vtiles[j % 3]
                nc.sync.dma_start(
                    out=v_dst[:].rearrange("p (c s) h d -> p c s h d", c=N_CORES),
                    in_=agv_out[j].rearrange("c p s h d -> p c s h d"),
                )

        # ---- attention per pair ----
        units = [(kb, h) for kb in range(NKB) for h in range(2)]  # 64 units
        nslots = (len(units) + 2) // 3  # 22

        def _attention(j, kt_cur, v_cur, prev_tail, mid=None):
            pO1 = psO.tile([DHEAD + 1, NQ], F32, tag="po1", name=f"pO1_{j}")
            pO2 = psO.tile([DHEAD + 1, NQ], F32, tag="po2", name=f"pO2_{j}")
            pOs = (pO1, pO2)
            slots_e = [None] * nslots

            def emit_scores(s):
                su = units[3 * s:3 * s + 3]
                ps = ring.tile([128, 3, NQ], F32, tag="ps", name=f"sc{j}_{s}")
                for i, (kb, h) in enumerate(su):
                    nc.tensor.matmul(
                        ps[:, i, :],
                        kt_cur[64 * h:64 * (h + 1), kb * 128:(kb + 1) * 128],
                        qT_sb[j][64 * h:64 * (h + 1), :],
                        start=True, stop=True,
                    )
                e = epool.tile([128, 3, NQ], BF16, tag="e", name=f"e{j}_{s}")
                nc.scalar.activation(e[:, 0:len(su), :], ps[:, 0:len(su), :], Exp)
                slots_e[s] = e

            def emit_av(s):
                su = units[3 * s:3 * s + 3]
                e = slots_e[s]
                for i, (kb, h) in enumerate(su):
                    nc.tensor.matmul(
                        pOs[h][:], v_cur[:, kb, h, :], e[:, i, :],
                        start=(kb == 0), stop=(kb == NKB - 1),
                    )

            hook = pre_slot_hooks.get(j)
            # 2-slot batches: scores(s), scores(s+1), then av(s-3), av(s-2)
            for s0 in range(0, nslots, 2):
                for s in (s0, s0 + 1):
                    if s >= nslots:
                        break
                    if hook is not None:
                        hook(s)
                    if s == 8 and mid is not None:
                        mid()
                    if j == 0 and s < 8:
                        with tc.high_priority():
                            emit_scores(s)
                    else:
                        emit_scores(s)
                if s0 == 0 and prev_tail:
                    while prev_tail:
                        prev_tail.pop(0)()
                for s in (s0 - 2, s0 - 1):
                    if s >= 0:
                        emit_av(s)

            def _normalize():
                den1 = npool.tile([1, NQ], F32, tag="den1", name=f"den1_{j}")
                den2 = npool.tile([1, NQ], F32, tag="den2", name=f"den2_{j}")
                nc.vector.tensor_copy(den1[:], pO1[DHEAD:DHEAD + 1, :])
                nc.vector.tensor_copy(den2[:], pO2[DHEAD:DHEAD + 1, :])
                rec1 = npool.tile([1, NQ], F32, tag="rec1", name=f"rec1_{j}")
                rec2 = npool.tile([1, NQ], F32, tag="rec2", name=f"rec2_{j}")
                nc.vector.reciprocal_approx_fast(rec1[:], den1[:])
                nc.vector.reciprocal_approx_fast(rec2[:], den2[:])
                b1 = npool.tile([DHEAD, NQ], F32, tag="b1", name=f"b1_{j}")
                b2 = npool.tile([DHEAD, NQ], F32, tag="b2", name=f"b2_{j}")
                nc.gpsimd.partition_broadcast(b1[:], rec1[:])
                nc.gpsimd.partition_broadcast(b2[:], rec2[:])
                nc.vector.tensor_mul(proj_sb[j][0:DHEAD, :], pO1[0:DHEAD, :], b1[:])
                nc.vector.tensor_mul(proj_sb[j][DHEAD:128, :], pO2[0:DHEAD, :], b2[:])

            return [lambda: emit_av(nslots - 2), lambda: emit_av(nslots - 1),
                    _normalize]

        tail = []
        for j in range(NPAIRS):
            # pair j+1's gather loads are emitted mid-pair (slot 8), after
            # the hook-drained AG producers for that pair are out.
            mid = (lambda jn=j + 1: _load_pair(jn)) if 1 <= j + 1 <= 5 else None
            kt_cur = ktiles[j % 2]
            v_cur = vtiles[j % 3]
            tail = _attention(j, kt_cur, v_cur, tail, mid=mid)
        while tail:
            tail.pop(0)()

        # ---- output projection: pair 5 + precomputed pairs 0-4 ----
        for qt in range(NQ // 128):
            ps = ring.tile([128, 3, NQ], F32, tag="ps", name=f"psf{qt}")
            lhs = proj_sb[5][:, qt * 128:(qt + 1) * 128]
            nc.tensor.matmul(ps[:, 0, :], lhs, wo_sb[5][:, 0:512],
                             start=True, stop=True)
            nc.tensor.matmul(ps[:, 1, 0:256], lhs, wo_sb[5][:, 512:DIM],
                             start=True, stop=True)
            nc.vector.tensor_add(accC[qt][:, 0:512], ps[:, 0, :], accC[qt][:, 0:512])
            nc.vector.tensor_add(accC[qt][:, 512:DIM], ps[:, 1, 0:256],
                                 accC[qt][:, 512:DIM])
            # two contiguous halves on separate queues halve the last store
            nc.sync.dma_start(out=out[qt * 128:qt * 128 + 64, :], in_=accC[qt][0:64, :])
            nc.scalar.dma_start(out=out[qt * 128 + 64:(qt + 1) * 128, :],
                                in_=accC[qt][64:128, :])

    nc.compile()
    return nc


def kernel(x, W_qkv, W_out, b_out):
    import ml_dtypes

    if "nc" not in _CACHE:
        _CACHE["nc"] = _build()
    nc = _CACHE["nc"]

    x = np.asarray(x, dtype=np.float32)
    W_qkv = np.asarray(W_qkv, dtype=np.float32)
    W_out = np.asarray(W_out, dtype=np.float32)
    b_out = np.asarray(b_out, dtype=np.float32)

    wq_h = np.ascontiguousarray(W_qkv[:, 0:DIM].astype(np.float16)).reshape(KT, 128, DIM)
    wk_h = np.ascontiguousarray(W_qkv[:, DIM:2 * DIM].astype(np.float16)).reshape(KT, 128, DIM)
    wv_h = np.ascontiguousarray(W_qkv[:, 2 * DIM:3 * DIM].astype(np.float16)).reshape(KT, 128, DIM)
    wo_h = np.ascontiguousarray(W_out.astype(ml_dtypes.bfloat16)).reshape(NPAIRS, 128, DIM)
    xT16_h = np.ascontiguousarray(x[0].T.astype(np.float16)).reshape(KT, 128, SEQ)

    in_maps = []
    for c in range(N_CORES):
        xc16_h = np.ascontiguousarray(
            x[0, c * NQ:(c + 1) * NQ, :].T.astype(np.float16)
        ).reshape(KT, 128, NQ)
        in_maps.append({
            "xc16": xc16_h, "xT16": xT16_h,
            "wq": wq_h, "wk": wk_h, "wv": wv_h,
            "wo": wo_h, "bo": b_out,
        })

    res = run_bass_kernel_spmd(
        nc, in_maps, list(range(N_CORES)),
        trace=bool(os.environ.get("KERNEL_TRACE")),
    )
    _CACHE["last_exec_time_ns"] = res.exec_time_ns
    out = np.concatenate([res.results[c]["out"] for c in range(N_CORES)], axis=0)
    return out.reshape(1, SEQ, DIM)
